# revision 1
# baseline (speedup 1.0000x reference)
"""BiDAF attention-flow kernel for Trainium2 (8 NeuronCores, data-parallel over batch).

Per core (one batch element):
  s[j,i]   = c[j] + q[i] + sum_h w_cq[h]*emb2[j,h]*emb1[i,h]
  a        = softmax_i(s)          (c[j] drops out of the row softmax)
  y2x      = a @ emb1
  b_att    = softmax_j(max_i s)
  x2y      = sum_j b_att[j]*emb2[j]
  out      = [emb2, y2x, emb2*y2x, emb2*x2y] @ w_red + b_red

Implementation notes:
  - b_c/b_q/b_cq cancel exactly in both softmaxes (row/column constants).
  - The row softmax uses a FIXED exp shift (s - SHIFT) instead of a row max:
    softmax is shift-invariant and fp32/bf16 exponent range absorbs the
    spread.  The true row max (needed for b_att) is recovered afterwards as
    SHIFT + ln(max_i u).  This removes the matmul->rowmax->exp serial chain.
  - y2x normalization (1/Z_j) is applied to the *output* psum of the
    reduction matmul blocks 2,3 (per-partition scalar in natural layout).
  - block1 + block4 = emb2 @ (w1 + x2y*w4): merged into one pass-2 matmul.
  - All bf16 transposes (emb1^T, emb2^T, u^T) run on the DMA transpose
    crossbar, keeping PE free for matmuls.
"""

import numpy as np
import ml_dtypes

P = 128
XL = 2048
YL = 2048
H = 768
OUT = 300
NJT = YL // P   # 16 j tiles
NIC = XL // P   # 16 i chunks
NHC = H // P    # 6 h chunks
SLAB = 512
NSLAB = XL // SLAB  # 4
NCORES = 8
SHIFT = 10.0    # fixed exp shift; |s| stays well below this + fp range

_CACHE = {}


def _fix_waits(nc, mybir, max_waits=1):
    """This walrus build rejects >1 sync wait per instruction.

    Pass 1: drop waits that are transitively implied by another wait on the
    same instruction (happens-before over per-engine / per-DMA-queue in-order
    streams plus wait edges).  Pass 2: hoist remaining extra waits onto
    same-engine NoOps inserted right before the instruction (for an in-order
    engine this blocks identically; DMA triggers are all on SP here and their
    awaited DMAs are always triggered earlier, so no cycles arise).
    """
    from collections import defaultdict

    blocks = [bb for f in nc.m.functions for bb in f.blocks]
    insts = [ins for bb in blocks for ins in bb.instructions]

    dma_types = ("InstDMACopy", "InstDmaTransposeAnt")
    eng_stream = defaultdict(list)
    queue_stream = defaultdict(list)
    sem_events = defaultdict(list)
    cum = defaultdict(int)
    for i, ins in enumerate(insts):
        eng_stream[str(ins.engine)].append(i)
        si = ins.sync_info
        if si and si.on_update:
            for u in si.on_update:
                cum[u.id] += u.update_value
                sem_events[u.id].append((cum[u.id], i))
                if type(ins).__name__ in dma_types:
                    queue_stream[u.id].append(i)

    def achiever(sem_id, val):
        for cv, i in sem_events.get(sem_id, []):
            if cv >= val:
                return i
        return None

    eng_pos, q_pos = {}, {}
    for e, lst in eng_stream.items():
        for k, i in enumerate(lst):
            eng_pos[i] = (e, k)
    for s, lst in queue_stream.items():
        for k, i in enumerate(lst):
            q_pos[i] = (s, k)

    memo = {}

    def implied(i):
        if i in memo:
            return memo[i]
        memo[i] = set()
        out = {i}
        ins = insts[i]
        if i in q_pos:
            s, k = q_pos[i]
            if k > 0:
                out |= implied(queue_stream[s][k - 1])
        e, k = eng_pos[i]
        j = k - 1
        while j >= 0:
            p = eng_stream[e][j]
            if type(insts[p]).__name__ in dma_types:
                j -= 1
                continue
            out |= implied(p)
            break
        si = ins.sync_info
        if si and si.on_wait:
            for w in si.on_wait:
                a = achiever(w.id, w.wait_value)
                if a is not None:
                    out |= implied(a)
        memo[i] = out
        return out

    # pass 1: redundancy elimination
    for i, ins in enumerate(insts):
        si = ins.sync_info
        if not (si and si.on_wait and len(si.on_wait) > max_waits):
            continue
        waits = list(si.on_wait)
        ach = [(w, achiever(w.id, w.wait_value)) for w in waits]
        keep = []
        for wi, (w, a) in enumerate(ach):
            red = False
            if a is not None:
                for wj, (w2, a2) in enumerate(ach):
                    if wi != wj and a2 is not None and a != a2 and a in implied(a2):
                        red = True
                        break
            if not red:
                keep.append(w)
        si.on_wait = keep

    # pass 2: hoist extras onto same-engine NoOps
    k = 0
    for bb in blocks:
        lst = bb.instructions
        i = 0
        while i < len(lst):
            ins = lst[i]
            si = ins.sync_info
            if si and si.on_wait and len(si.on_wait) > max_waits:
                waits = list(si.on_wait)
                extra, keep = waits[:-max_waits], waits[-max_waits:]
                si.on_wait = keep
                nops = []
                for w in extra:
                    nop = mybir.InstNoOp(name=f"I-waitfix-{k}", ins=[], outs=[])
                    k += 1
                    nop.engine = ins.engine
                    nop.sync_info = mybir.SyncInfo(on_wait=[w], on_update=[])
                    nops.append(nop)
                lst[i:i] = nops
                i += len(nops)
            i += 1


def _build():
    import concourse.bass as bass
    import concourse.tile as tile
    import concourse.mybir as mybir
    from concourse.masks import make_identity

    f32 = mybir.dt.float32
    f32r = mybir.dt.float32r
    bf16 = mybir.dt.bfloat16
    MUL = mybir.AluOpType.mult
    ADD = mybir.AluOpType.add
    MAX = mybir.AluOpType.max
    EXP = mybir.ActivationFunctionType.Exp
    LN = mybir.ActivationFunctionType.Ln
    AXX = mybir.AxisListType.X

    nc = bass.Bass("TRN2", target_bir_lowering=False, debug=False,
                   num_devices=NCORES)

    emb1_d = nc.dram_tensor("emb1", [XL, H], bf16, kind="ExternalInput")
    emb2_d = nc.dram_tensor("emb2", [YL, H], bf16, kind="ExternalInput")
    wc_d = nc.dram_tensor("wc", [P, NHC], bf16, kind="ExternalInput")
    wq_d = nc.dram_tensor("wq", [P, NHC], bf16, kind="ExternalInput")
    wcq_d = nc.dram_tensor("wcq", [P, NHC], f32, kind="ExternalInput")
    w1_d = nc.dram_tensor("w1", [H, OUT], f32, kind="ExternalInput")
    w2_d = nc.dram_tensor("w2", [H, OUT], bf16, kind="ExternalInput")
    w3_d = nc.dram_tensor("w3", [H, OUT], bf16, kind="ExternalInput")
    w4_d = nc.dram_tensor("w4", [H, OUT], f32, kind="ExternalInput")
    bred_d = nc.dram_tensor("bred", [1, OUT], f32, kind="ExternalInput")
    out_d = nc.dram_tensor("out", [YL, OUT], f32, kind="ExternalOutput")

    with tile.TileContext(nc) as tc:
        with (
            tc.tile_pool(name="res", bufs=1) as res,        # resident data
            tc.tile_pool(name="stage", bufs=3) as stage,    # dma staging
            tc.tile_pool(name="small", bufs=1) as small,    # stats etc
            tc.tile_pool(name="pst", bufs=2, space="PSUM") as pst,
            tc.tile_pool(name="pss", bufs=2, space="PSUM") as pss,
            tc.tile_pool(name="psy", bufs=1, space="PSUM") as psy,
            tc.tile_pool(name="pso", bufs=2, space="PSUM") as pso,
            tc.tile_pool(name="dpool", bufs=1, space="DRAM") as dpool,
        ):
            # ---- constants / weights ----
            ident16 = res.tile([P, P], bf16, tag="ident16")
            make_identity(nc, ident16)
            ident32 = res.tile([P, P], f32, tag="ident32")
            make_identity(nc, ident32)
            ones32 = res.tile([1, P], f32, tag="ones32")
            nc.vector.memset(ones32, 1.0)
            ones_r = res.tile([1, P], f32r, tag="ones_r")
            nc.vector.tensor_copy(out=ones_r, in_=ones32)
            negC = res.tile([P, 1], f32, tag="negC")
            nc.vector.memset(negC, -SHIFT)

            # PE warm-up: keep the HAM activity monitor busy while the input
            # DMAs stream in, so the clock is at 2.4 GHz when the real
            # matmuls start.  No data deps; results are discarded.
            for wk in range(220):
                wps = pss.tile([P, P], bf16, tag="pss", name=f"warm{wk}")
                nc.tensor.transpose(wps, ident16, ident16)

            wc_sb = res.tile([P, NHC], bf16, tag="wc")
            nc.sync.dma_start(out=wc_sb, in_=wc_d[:])
            wq_sb = res.tile([P, NHC], bf16, tag="wq")
            nc.sync.dma_start(out=wq_sb, in_=wq_d[:])
            wcq_sb = res.tile([P, NHC], f32, tag="wcq")
            nc.sync.dma_start(out=wcq_sb, in_=wcq_d[:])

            w1_sb = res.tile([P, NHC, OUT], f32, tag="w1")
            w2_sb = res.tile([P, NHC, OUT], bf16, tag="w2")
            w3_sb = res.tile([P, NHC, OUT], bf16, tag="w3")
            w4_sb = res.tile([P, NHC, OUT], f32, tag="w4")
            for hc in range(NHC):
                nc.sync.dma_start(out=w1_sb[:, hc, :], in_=w1_d[hc * P:(hc + 1) * P, :])
                nc.sync.dma_start(out=w2_sb[:, hc, :], in_=w2_d[hc * P:(hc + 1) * P, :])
                nc.sync.dma_start(out=w3_sb[:, hc, :], in_=w3_d[hc * P:(hc + 1) * P, :])
                nc.sync.dma_start(out=w4_sb[:, hc, :], in_=w4_d[hc * P:(hc + 1) * P, :])
            bred_bc = res.tile([P, OUT], f32, tag="bred_bc")
            _bap = bred_d.ap()
            nc.sync.dma_start(out=bred_bc, in_=bass.AP(
                tensor=_bap.tensor, offset=_bap.offset,
                ap=[[0, P]] + list(_bap.ap[1:])))

            # ---- resident embeddings ----
            # e1n: natural (i, h) bf16; e1tt: emb1^T as (h_in, hc, i) bf16
            # e2tt: emb2^T as (h_in, hc, j) bf16; e2ts: e2tt * w_cq
            e1n = [res.tile([P, H], bf16, tag=f"e1n{i}", name=f"e1n{i}")
                   for i in range(NIC)]
            e1tt = res.tile([P, NHC, XL], bf16, tag="e1tt")
            e2tt = res.tile([P, NHC, YL], bf16, tag="e2tt")
            e2ts = res.tile([P, NHC, YL], bf16, tag="e2ts")

            def load_e2_chunk(jc):
                jsl = slice(jc * P, (jc + 1) * P)
                st = stage.tile([P, H], bf16, tag="stage", name=f"e2st{jc}")
                nc.sync.dma_start(out=st, in_=emb2_d[jsl, :])
                for hc in range(NHC):
                    ps = pst.tile([P, P], bf16, tag="pst", name=f"e2ps{jc}_{hc}")
                    nc.tensor.transpose(ps, st[:, hc * P:(hc + 1) * P], ident16)
                    nc.any.tensor_copy(out=e2tt[:, hc, jsl], in_=ps)
                    nc.vector.tensor_scalar_mul(
                        e2ts[:, hc, jsl], ps, wcq_sb[:, hc:hc + 1])

            def load_e1_chunk(ic):
                isl = slice(ic * P, (ic + 1) * P)
                nc.sync.dma_start(out=e1n[ic], in_=emb1_d[isl, :])
                for hc in range(NHC):
                    ps = pst.tile([P, P], bf16, tag="pst", name=f"e1ps{ic}_{hc}")
                    nc.tensor.transpose(ps, e1n[ic][:, hc * P:(hc + 1) * P],
                                        ident16)
                    nc.any.tensor_copy(out=e1tt[:, hc, isl], in_=ps)

            load_e2_chunk(0)
            for ic in range(NIC):
                load_e1_chunk(ic)
            for jc in range(1, NJT):
                load_e2_chunk(jc)

            # ---- q_row = emb1 @ w_q as a (1, XL) row ----
            q_row = small.tile([1, XL], f32, tag="q_row")
            for sl in range(NSLAB):
                ssl = slice(sl * SLAB, (sl + 1) * SLAB)
                qp = pst.tile([1, SLAB], f32, tag="pst", name=f"qp{sl}")
                for hc in range(NHC):
                    nc.tensor.matmul(
                        qp, wq_sb[:, hc:hc + 1], e1tt[:, hc, ssl],
                        start=(hc == 0), stop=(hc == NHC - 1),
                        skip_group_check=True)
                nc.any.tensor_copy(out=q_row[:, ssl].bitcast(f32r), in_=qp)

            # c_row = emb2 @ w_c as a (1, YL) row (wc stationary), then
            # reshape to (P, NJT) columns via a DRAM bounce
            c_rowf = small.tile([1, YL], f32, tag="c_rowf")
            for sl in range(NSLAB):
                ssl = slice(sl * SLAB, (sl + 1) * SLAB)
                cp = pst.tile([1, SLAB], f32, tag="pst", name=f"cp{sl}")
                for hc in range(NHC):
                    nc.tensor.matmul(
                        cp, wc_sb[:, hc:hc + 1], e2tt[:, hc, ssl],
                        start=(hc == 0), stop=(hc == NHC - 1),
                        skip_group_check=True)
                nc.any.tensor_copy(out=c_rowf[:, ssl], in_=cp)
            crd = dpool.tile([1, YL], f32, tag="crd")
            nc.sync.dma_start(out=crd, in_=c_rowf)
            c_sb = small.tile([P, NJT], f32, tag="c_sb")
            nc.sync.dma_start(out=c_sb, in_=bass.AP(
                tensor=crd.tensor, offset=crd.offset, ap=[[1, P], [P, NJT]]))

            # ---- stats tiles ----
            M_sb = small.tile([P, NJT], f32, tag="M")
            Z_sb = small.tile([P, NJT], f32, tag="Z")
            rZ_sb = small.tile([P, NJT], f32, tag="rZ")
            out_sb = res.tile([P, NJT, OUT], f32, tag="out_sb")

            # ---- main loop over j tiles ----
            sjt_cm = tc.tile_pool(name="sjt", bufs=2)
            sjt = sjt_cm.__enter__()
            for jt in range(NJT):
                jsl = slice(jt * P, (jt + 1) * P)

                # s = q + (emb2*wcq) @ emb1^T; u = exp(s - SHIFT) slab by slab
                u = sjt.tile([P, XL], bf16, tag="u", name=f"u{jt}")
                Zp = sjt.tile([P, NSLAB], f32, tag="Zp", name=f"Zp{jt}")
                for sl in range(NSLAB):
                    ssl = slice(sl * SLAB, (sl + 1) * SLAB)
                    sp = pss.tile([P, SLAB], f32, tag="pss", name=f"sp{jt}_{sl}")
                    nc.tensor.matmul(sp, ones_r, q_row[:, ssl].bitcast(f32r),
                                     start=True, stop=False,
                                     skip_group_check=True)
                    for hc in range(NHC):
                        nc.tensor.matmul(
                            sp, e2ts[:, hc, jsl], e1tt[:, hc, ssl],
                            start=False, stop=(hc == NHC - 1),
                            skip_group_check=True)
                    nc.scalar.activation(out=u[:, ssl], in_=sp, func=EXP,
                                         bias=negC, scale=1.0,
                                         accum_out=Zp[:, sl:sl + 1])
                nc.vector.tensor_reduce(out=Z_sb[:, jt:jt + 1], in_=Zp,
                                        axis=AXX, op=ADD)
                nc.vector.reciprocal(out=rZ_sb[:, jt:jt + 1],
                                     in_=Z_sb[:, jt:jt + 1])

                # row max for b_att: M = c + SHIFT + ln(max u)
                umax = sjt.tile([P, 1], f32, tag="umax", name=f"umax{jt}")
                nc.vector.tensor_reduce(out=umax, in_=u, axis=AXX, op=MAX)
                lnu = sjt.tile([P, 1], f32, tag="lnu", name=f"lnu{jt}")
                nc.scalar.activation(out=lnu, in_=umax, func=LN)
                nc.vector.scalar_tensor_tensor(
                    out=M_sb[:, jt:jt + 1], in0=lnu, scalar=SHIFT,
                    in1=c_sb[:, jt:jt + 1], op0=ADD, op1=ADD)

                # u^T via PE transposes, batched 4 blocks per psum tile so
                # the psum->sbuf copies are wide and don't lockstep with PE
                uT = sjt.tile([P, NIC, P], bf16, tag="uT", name=f"uT{jt}")
                for g in range(NIC // 4):
                    tp = pss.tile([P, 4, P], bf16, tag="pss", name=f"tp{jt}_{g}")
                    for k in range(4):
                        ic = g * 4 + k
                        nc.tensor.transpose(tp[:, k, :],
                                            u[:, ic * P:(ic + 1) * P], ident16)
                    nc.any.tensor_copy(out=uT[:, g * 4:(g + 1) * 4, :], in_=tp)

                # y2x_unnorm^T = emb1(natural-as-lhsT) @ uT
                yps = psy.tile([P, NHC, P], f32, tag="psy", name=f"yps{jt}")
                for hc in range(NHC):
                    for ic in range(NIC):
                        nc.tensor.matmul(
                            yps[:, hc, :], e1n[ic][:, hc * P:(hc + 1) * P],
                            uT[:, ic, :],
                            start=(ic == 0), stop=(ic == NIC - 1))

                y2xT = sjt.tile([P, NHC, P], bf16, tag="y2xT", name=f"y2xT{jt}")
                bl3 = sjt.tile([P, NHC, P], bf16, tag="bl3", name=f"bl3{jt}")
                for hc in range(NHC):
                    nc.vector.tensor_copy(out=y2xT[:, hc, :], in_=yps[:, hc, :])
                    nc.vector.tensor_mul(bl3[:, hc, :], e2tt[:, hc, jsl],
                                         y2xT[:, hc, :])

                # pass-1 reduction: [y2x; e2*y2x] @ [w2; w3]
                op1 = pso.tile([P, OUT], f32, tag="pso", name=f"op1_{jt}")
                for hc in range(NHC):
                    nc.tensor.matmul(op1, y2xT[:, hc, :], w2_sb[:, hc, :],
                                     start=(hc == 0), stop=False,
                                     skip_group_check=True)
                for hc in range(NHC):
                    nc.tensor.matmul(op1, bl3[:, hc, :], w3_sb[:, hc, :],
                                     start=False, stop=(hc == NHC - 1),
                                     skip_group_check=True)
                # out_sb = psum/Z + b_red
                nc.vector.scalar_tensor_tensor(
                    out=out_sb[:, jt, :], in0=op1, scalar=rZ_sb[:, jt:jt + 1],
                    in1=bred_bc, op0=MUL, op1=ADD)

            sjt_cm.__exit__(None, None, None)
            post_cm = tc.tile_pool(name="post", bufs=1)
            post = post_cm.__enter__()

            # ---- b_att = softmax_j(M) ----
            # global max over partitions via PE transpose + free-dim reduce,
            # then broadcast back with a K=1 matmul against a ones row.
            mx = post.tile([P, 1], f32, tag="mx")
            nc.vector.tensor_reduce(out=mx, in_=M_sb, axis=AXX, op=MAX)
            tpm = pst.tile([1, P], f32, tag="pst", name="tpm")
            nc.tensor.transpose(tpm, mx, ident32)
            mrow = post.tile([1, P], f32, tag="mrow")
            nc.vector.tensor_copy(out=mrow, in_=tpm)
            ng0 = post.tile([1, 1], f32, tag="ng0")
            nc.vector.tensor_reduce(out=ng0, in_=mrow, axis=AXX, op=MAX,
                                    negate=True)
            ngp = pst.tile([P, 1], f32, tag="pst", name="ngp")
            nc.tensor.matmul(ngp, ones32, ng0, start=True, stop=True,
                             skip_group_check=True)
            ngm = post.tile([P, 1], f32, tag="ngm")
            nc.vector.tensor_copy(out=ngm, in_=ngp)

            bexp = post.tile([P, NJT], f32, tag="bexp")
            brow = post.tile([P, 1], f32, tag="brow")
            nc.scalar.activation(out=bexp, in_=M_sb, func=EXP, bias=ngm,
                                 scale=1.0, accum_out=brow)
            tpb = pst.tile([1, P], f32, tag="pst", name="tpb")
            nc.tensor.transpose(tpb, brow, ident32)
            brw = post.tile([1, P], f32, tag="brw")
            nc.vector.tensor_copy(out=brw, in_=tpb)
            bs0 = post.tile([1, 1], f32, tag="bs0")
            nc.vector.tensor_reduce(out=bs0, in_=brw, axis=AXX, op=ADD)
            rb0 = post.tile([1, 1], f32, tag="rb0")
            nc.vector.reciprocal(rb0, bs0)
            rbp = pst.tile([P, 1], f32, tag="pst", name="rbp")
            nc.tensor.matmul(rbp, ones32, rb0, start=True, stop=True,
                             skip_group_check=True)
            rbz = post.tile([P, 1], f32, tag="rbz")
            nc.vector.tensor_copy(out=rbz, in_=rbp)
            batt = post.tile([P, NJT], bf16, tag="batt")
            nc.vector.tensor_scalar_mul(batt, bexp, rbz)

            # b_att as j-partition columns: transpose to (NJT, P), bounce
            # through a DRAM row, read back as (P, NJT) with a strided AP.
            btp = pst.tile([NJT, P], bf16, tag="pst", name="btp")
            nc.tensor.transpose(btp, batt, ident16)
            btmp = post.tile([NJT, P], bf16, tag="btmp")
            nc.vector.tensor_copy(out=btmp, in_=btp)
            scrd = dpool.tile([1, YL], bf16, tag="scrd")
            nc.sync.dma_start(out=scrd, in_=btmp)
            battjp = post.tile([P, NJT], bf16, tag="battjp")
            nc.sync.dma_start(out=battjp, in_=bass.AP(
                tensor=scrd.tensor, offset=scrd.offset, ap=[[1, P], [P, NJT]]))

            # x2y = sum_j b_att[j]*emb2[j]: PE matmuls with the b_att column
            # as a 1-wide stationary operand against natural emb2 chunks
            # re-read from DRAM (bf16), accumulated over j chunks.
            e2nt = [post.tile([P, H], bf16, tag=f"e2n{jc}", name=f"e2n{jc}")
                    for jc in range(NJT)]
            for jc in range(NJT):
                nc.sync.dma_start(out=e2nt[jc], in_=emb2_d[jc * P:(jc + 1) * P, :])
            x2p = psy.tile([1, H], f32, tag="psy", name="x2p")
            for hsl in (slice(0, 512), slice(512, H)):
                for jc in range(NJT):
                    nc.tensor.matmul(
                        x2p[:, hsl], battjp[:, jc:jc + 1], e2nt[jc][:, hsl],
                        start=(jc == 0), stop=(jc == NJT - 1),
                        skip_group_check=True)
            x2row = post.tile([1, H], f32, tag="x2row")
            nc.any.tensor_copy(out=x2row, in_=x2p)
            x2d = dpool.tile([1, H], f32, tag="x2d")
            nc.sync.dma_start(out=x2d, in_=x2row)
            x2yT = post.tile([P, NHC], f32, tag="x2yT")
            nc.sync.dma_start(out=x2yT, in_=bass.AP(
                tensor=x2d.tensor, offset=x2d.offset, ap=[[1, P], [P, NHC]]))

            # w14' = w1 + x2y*w4
            w14 = res.tile([P, NHC, OUT], bf16, tag="w14")
            for hc in range(NHC):
                nc.vector.scalar_tensor_tensor(
                    out=w14[:, hc, :], in0=w4_sb[:, hc, :],
                    scalar=x2yT[:, hc:hc + 1], in1=w1_sb[:, hc, :],
                    op0=MUL, op1=ADD)

            # ---- pass 2: out += emb2 @ w14' ----
            for jt in range(NJT):
                jsl = slice(jt * P, (jt + 1) * P)
                op2 = pso.tile([P, OUT], f32, tag="pso", name=f"op2_{jt}")
                for hc in range(NHC):
                    nc.tensor.matmul(op2, e2tt[:, hc, jsl], w14[:, hc, :],
                                     start=(hc == 0), stop=(hc == NHC - 1),
                                     skip_group_check=True)
                fin = stage.tile([P, OUT], f32, tag="fin", name=f"fin{jt}")
                nc.vector.tensor_add(fin, op2, out_sb[:, jt, :])
                nc.sync.dma_start(out=out_d[jsl, :], in_=fin)
            post_cm.__exit__(None, None, None)

    return nc


def _get_nc(drain_fix=True):
    if "nc" not in _CACHE:
        _CACHE["nc"] = _build()
    if drain_fix and not _CACHE.get("drain_fixed"):
        import concourse.mybir as mybir
        _fix_waits(_CACHE["nc"], mybir, max_waits=1)
        _CACHE["drain_fixed"] = True
    return _CACHE["nc"]


def kernel(emb1, emb2, w_c, b_c, w_q, b_q, w_cq, b_cq, w_red, b_red):
    from concourse.bass_utils import run_bass_kernel_spmd

    nc = _get_nc()
    bf = ml_dtypes.bfloat16

    emb1 = np.ascontiguousarray(np.asarray(emb1, dtype=np.float32).astype(bf))
    emb2 = np.ascontiguousarray(np.asarray(emb2, dtype=np.float32).astype(bf))
    w_red = np.asarray(w_red, dtype=np.float32)

    # b_c, b_q, b_cq cancel exactly in both softmaxes (per-row/col consts).
    wc = np.ascontiguousarray(np.asarray(w_c, np.float32).reshape(NHC, P).T.astype(bf))
    wq = np.ascontiguousarray(np.asarray(w_q, np.float32).reshape(NHC, P).T.astype(bf))
    wcq = np.ascontiguousarray(np.asarray(w_cq, np.float32).reshape(NHC, P).T)
    w1 = np.ascontiguousarray(w_red[0:H])
    w2 = np.ascontiguousarray(w_red[H:2 * H].astype(bf))
    w3 = np.ascontiguousarray(w_red[2 * H:3 * H].astype(bf))
    w4 = np.ascontiguousarray(w_red[3 * H:4 * H])
    bred = np.ascontiguousarray(np.asarray(b_red, np.float32).reshape(1, OUT))

    in_maps = []
    for b in range(NCORES):
        in_maps.append({
            "emb1": emb1[b], "emb2": emb2[b],
            "wc": wc, "wq": wq, "wcq": wcq,
            "w1": w1, "w2": w2, "w3": w3, "w4": w4, "bred": bred,
        })
    res = run_bass_kernel_spmd(nc, in_maps, core_ids=list(range(NCORES)))
    return np.stack([res.results[i]["out"] for i in range(NCORES)], axis=0)



# revision 6
# speedup vs baseline: 1.5915x; 1.5915x over previous
"""BiDAF attention-flow kernel for Trainium2 (8 NeuronCores, data-parallel over batch).

Per core (one batch element):
  s[j,i]   = c[j] + q[i] + sum_h w_cq[h]*emb2[j,h]*emb1[i,h]
  a        = softmax_i(s)          (c[j] drops out of the row softmax)
  y2x      = a @ emb1
  b_att    = softmax_j(max_i s)
  x2y      = sum_j b_att[j]*emb2[j]
  out      = [emb2, y2x, emb2*y2x, emb2*x2y] @ w_red + b_red

Implementation notes (v2 restructure):
  - Host precomputes: (emb1*w_cq)^T bf16, emb2^T bf16, emb1 bf16, the tiny
    row/col scores q = emb1@w_q and ec = exp(c - max c), all exact in f32.
    No load transposes or score matmuls on device.
  - s-phase psum is PRE-INITIALIZED with a q broadcast by the scalar engine
    (matmul start=False accumulates on top) -- no seed matmuls.
  - exp uses a FIXED shift (softmax shift-invariance); accum_out gives Z.
    Row softmax a = u/Z is applied to u BEFORE the y2x matmul, which lets
    the whole reduction run as ONE fused psum group per j-tile.
  - b_att = normalize_j(rowmax(u) * ec): no ln/exp round trip needed.
  - u^T comes from ONE DMA-crossbar transpose per j-tile (3D out AP maps
    source col i to partition i%128, block i//128); PE does only matmuls.
  - x2y is computed on the vector engine as sum_j(e2^T * b_row) with b_row
    partition-broadcast via a DRAM bounce; 1/sum(b) is folded into w14.
  - y2x runs as an all-j phase with N=512 moving operands (4 j-window x
    6 h-chunk psum groups, 16 accumulating matmuls each).
  - pass: out = [a@e1]@w2 + [e2*(a@e1)]@w3 + e2@(w1 + x2y*w4) fused into a
    single 18-matmul psum group per j-tile.
"""

import numpy as np
import ml_dtypes

P = 128
XL = 2048
YL = 2048
H = 768
OUT = 300
NJT = YL // P   # 16 j tiles
NIC = XL // P   # 16 i chunks
NHC = H // P    # 6 h chunks
SLAB = 512
NSLAB = XL // SLAB  # 4
NJW = YL // 512     # 4 j windows for y2x
NCORES = 8
SHIFT = 10.0    # fixed exp shift; |s| stays well below this + fp range
WARM = 220      # PE warm-up transposes (DVFS ramp) while inputs stream in

_CACHE = {}


def _fix_waits(nc, mybir, max_waits=1):
    """This walrus build rejects >1 sync wait per instruction.

    Pass 1: drop waits that are transitively implied by another wait on the
    same instruction (happens-before over per-engine / per-DMA-queue in-order
    streams plus wait edges).  Pass 2: hoist remaining extra waits onto
    same-engine NoOps inserted right before the instruction (for an in-order
    engine this blocks identically; DMA triggers are all on SP/Act here and
    their awaited DMAs are always triggered earlier, so no cycles arise).
    """
    from collections import defaultdict

    blocks = [bb for f in nc.m.functions for bb in f.blocks]
    insts = [ins for bb in blocks for ins in bb.instructions]

    dma_types = ("InstDMACopy", "InstDmaTransposeAnt")
    eng_stream = defaultdict(list)
    queue_stream = defaultdict(list)
    sem_events = defaultdict(list)
    cum = defaultdict(int)
    for i, ins in enumerate(insts):
        eng_stream[str(ins.engine)].append(i)
        si = ins.sync_info
        if si and si.on_update:
            for u in si.on_update:
                cum[u.id] += u.update_value
                sem_events[u.id].append((cum[u.id], i))
                if type(ins).__name__ in dma_types:
                    queue_stream[u.id].append(i)

    def achiever(sem_id, val):
        for cv, i in sem_events.get(sem_id, []):
            if cv >= val:
                return i
        return None

    eng_pos, q_pos = {}, {}
    for e, lst in eng_stream.items():
        for k, i in enumerate(lst):
            eng_pos[i] = (e, k)
    for s, lst in queue_stream.items():
        for k, i in enumerate(lst):
            q_pos[i] = (s, k)

    memo = {}

    def implied(i):
        if i in memo:
            return memo[i]
        memo[i] = set()
        out = {i}
        ins = insts[i]
        if i in q_pos:
            s, k = q_pos[i]
            if k > 0:
                out |= implied(queue_stream[s][k - 1])
        e, k = eng_pos[i]
        j = k - 1
        while j >= 0:
            p = eng_stream[e][j]
            if type(insts[p]).__name__ in dma_types:
                j -= 1
                continue
            out |= implied(p)
            break
        si = ins.sync_info
        if si and si.on_wait:
            for w in si.on_wait:
                a = achiever(w.id, w.wait_value)
                if a is not None:
                    out |= implied(a)
        memo[i] = out
        return out

    # pass 1: redundancy elimination
    for i, ins in enumerate(insts):
        si = ins.sync_info
        if not (si and si.on_wait and len(si.on_wait) > max_waits):
            continue
        waits = list(si.on_wait)
        ach = [(w, achiever(w.id, w.wait_value)) for w in waits]
        keep = []
        for wi, (w, a) in enumerate(ach):
            red = False
            if a is not None:
                for wj, (w2, a2) in enumerate(ach):
                    if wi != wj and a2 is not None and a != a2 and a in implied(a2):
                        red = True
                        break
            if not red:
                keep.append(w)
        si.on_wait = keep

    # pass 2: hoist extras onto same-engine NoOps
    k = 0
    for bb in blocks:
        lst = bb.instructions
        i = 0
        while i < len(lst):
            ins = lst[i]
            si = ins.sync_info
            if si and si.on_wait and len(si.on_wait) > max_waits:
                waits = list(si.on_wait)
                extra, keep = waits[:-max_waits], waits[-max_waits:]
                si.on_wait = keep
                nops = []
                for w in extra:
                    nop = mybir.InstNoOp(name=f"I-waitfix-{k}", ins=[], outs=[])
                    k += 1
                    nop.engine = ins.engine
                    nop.sync_info = mybir.SyncInfo(on_wait=[w], on_update=[])
                    nops.append(nop)
                lst[i:i] = nops
                i += len(nops)
            i += 1


def _build():
    import concourse.bass as bass
    import concourse.tile as tile
    import concourse.mybir as mybir
    from concourse.masks import make_identity

    f32 = mybir.dt.float32
    bf16 = mybir.dt.bfloat16
    MUL = mybir.AluOpType.mult
    ADD = mybir.AluOpType.add
    MAX = mybir.AluOpType.max
    EXP = mybir.ActivationFunctionType.Exp
    CPY = mybir.ActivationFunctionType.Copy
    AXX = mybir.AxisListType.X

    nc = bass.Bass("TRN2", target_bir_lowering=False, debug=False,
                   num_devices=NCORES)

    e1t_d = nc.dram_tensor("e1t", [H, XL], bf16, kind="ExternalInput")   # (emb1*wcq)^T
    e1n_d = nc.dram_tensor("e1n", [XL, H], bf16, kind="ExternalInput")   # emb1
    e2t_d = nc.dram_tensor("e2t", [H, YL], bf16, kind="ExternalInput")   # emb2^T
    q_d = nc.dram_tensor("q", [1, XL], f32, kind="ExternalInput")
    ec_d = nc.dram_tensor("ec", [P, NJT], f32, kind="ExternalInput")     # exp(c - cmax)
    w1_d = nc.dram_tensor("w1", [H, OUT], bf16, kind="ExternalInput")
    w2_d = nc.dram_tensor("w2", [H, OUT], bf16, kind="ExternalInput")
    w3_d = nc.dram_tensor("w3", [H, OUT], bf16, kind="ExternalInput")
    w4_d = nc.dram_tensor("w4", [H, OUT], bf16, kind="ExternalInput")
    bred_d = nc.dram_tensor("bred", [1, OUT], f32, kind="ExternalInput")
    out_d = nc.dram_tensor("out", [YL, OUT], f32, kind="ExternalOutput")

    def bcast_ap(dram_ap, n_part, off_elems, inner):
        return bass.AP(tensor=dram_ap.tensor, offset=dram_ap.offset + off_elems,
                       ap=[[0, n_part]] + inner)

    with tile.TileContext(nc) as tc:
        with (
            tc.tile_pool(name="res", bufs=1) as res,        # resident data
            tc.tile_pool(name="stage", bufs=3) as stage,    # out staging
            tc.tile_pool(name="small", bufs=1) as small,    # stats etc
            tc.tile_pool(name="pst", bufs=2, space="PSUM") as pst,
            tc.tile_pool(name="dpool", bufs=1, space="DRAM") as dpool,
        ):
            # ---- constants ----
            ident16 = res.tile([P, P], bf16, tag="ident16")
            make_identity(nc, ident16)
            ident32 = res.tile([P, P], f32, tag="ident32")
            make_identity(nc, ident32)
            ones32 = res.tile([1, P], f32, tag="ones32")
            nc.vector.memset(ones32, 1.0)
            negC = res.tile([P, 1], f32, tag="negC")
            nc.vector.memset(negC, -SHIFT)

            # PE warm-up for the DVFS/HAM ramp while inputs stream in.
            for wk in range(WARM):
                wps = pst.tile([P, P], bf16, tag="pst", name=f"warm{wk}")
                nc.tensor.transpose(wps, ident16, ident16)

            # ---- loads ----
            # SP queue: per-hc chunks of e2^T and (e1*wcq)^T, interleaved so
            # the s-phase can start after the first pair.
            e2tt = res.tile([P, NHC, YL], bf16, tag="e2tt")
            ph1_cm = tc.tile_pool(name="ph1", bufs=1)
            ph1 = ph1_cm.__enter__()
            pse_cm = tc.tile_pool(name="pse", bufs=3, space="PSUM")
            pse = pse_cm.__enter__()
            e1s = ph1.tile([P, NHC, XL], bf16, tag="e1s")
            _e2ap = e2t_d.ap()
            _e1ap = e1t_d.ap()
            for hc in range(NHC):
                nc.sync.dma_start(out=e2tt[:, hc, :], in_=bass.AP(
                    tensor=_e2ap.tensor, offset=hc * P * YL,
                    ap=[[YL, P], [1, YL]]))
                nc.sync.dma_start(out=e1s[:, hc, :], in_=bass.AP(
                    tensor=_e1ap.tensor, offset=hc * P * XL,
                    ap=[[XL, P], [1, XL]]))

            # Act queue: q broadcast (needed first), ec, bred, weights, e1n.
            q_bc = ph1.tile([P, NSLAB, SLAB], f32, tag="q_bc")
            _qap = q_d.ap()
            for sl in range(NSLAB):
                nc.scalar.dma_start(out=q_bc[:, sl, :],
                                    in_=bcast_ap(_qap, P, sl * SLAB,
                                                 [[1, SLAB]]))
            ec_sb = small.tile([P, NJT], f32, tag="ec")
            nc.scalar.dma_start(out=ec_sb, in_=ec_d[:])
            bred_bc = res.tile([P, OUT], f32, tag="bred_bc")
            nc.scalar.dma_start(out=bred_bc,
                                in_=bcast_ap(bred_d.ap(), P, 0, [[1, OUT]]))

            w1b = res.tile([P, NHC, OUT], bf16, tag="w1")
            w2b = res.tile([P, NHC, OUT], bf16, tag="w2")
            w3b = res.tile([P, NHC, OUT], bf16, tag="w3")
            w4b = res.tile([P, NHC, OUT], bf16, tag="w4")
            for wsb, wd in ((w1b, w1_d), (w2b, w2_d), (w3b, w3_d), (w4b, w4_d)):
                _wap = wd.ap()
                nc.scalar.dma_start(out=wsb, in_=bass.AP(
                    tensor=_wap.tensor, offset=0,
                    ap=[[OUT, P], [OUT * P, NHC], [1, OUT]]))

            # emb1 natural, i-chunk-major: e1nb[p, d, h] = emb1[d*128+p, h]
            e1nb = res.tile([P, NIC, H], bf16, tag="e1nb")
            _e1nap = e1n_d.ap()
            nc.scalar.dma_start(out=e1nb, in_=bass.AP(
                tensor=_e1nap.tensor, offset=0,
                ap=[[H, P], [H * P, NIC], [1, H]]))

            # ---- stats tiles ----
            m_sb = small.tile([P, NJT], f32, tag="m_sb")    # rowmax of u
            Z_sb = small.tile([P, NJT], f32, tag="Z_sb")
            rZ_sb = small.tile([P, NJT], f32, tag="rZ_sb")

            # u^T for ALL j: uTall[p, d, jt, jj] = a[jt*128+jj, d*128+p]
            uTall = res.tile([P, NIC, NJT, P], bf16, tag="uTall")

            # ---- phase 1: s matmuls + exp + normalize + transpose ----
            ph1u = tc.tile_pool(name="ph1u", bufs=2)
            ph1u_p = ph1u.__enter__()
            for jt in range(NJT):
                jsl = slice(jt * P, (jt + 1) * P)
                u = ph1u_p.tile([P, XL], bf16, tag="u", name=f"u{jt}")
                Zp = ph1u_p.tile([P, NSLAB], f32, tag="Zp", name=f"Zp{jt}")
                for sl in range(NSLAB):
                    ssl = slice(sl * SLAB, (sl + 1) * SLAB)
                    sp = pse.tile([P, SLAB], f32, tag="pse",
                                  name=f"sp{jt}_{sl}")
                    nc.scalar.activation(out=sp, in_=q_bc[:, sl, :], func=CPY)
                    for hc in range(NHC):
                        nc.tensor.matmul(
                            sp, e2tt[:, hc, jsl], e1s[:, hc, ssl],
                            start=False, stop=(hc == NHC - 1),
                            skip_group_check=True)
                    nc.scalar.activation(out=u[:, ssl], in_=sp, func=EXP,
                                         bias=negC, scale=1.0,
                                         accum_out=Zp[:, sl:sl + 1])
                nc.vector.tensor_reduce(out=m_sb[:, jt:jt + 1], in_=u,
                                        axis=AXX, op=MAX)
                nc.vector.tensor_reduce(out=Z_sb[:, jt:jt + 1], in_=Zp,
                                        axis=AXX, op=ADD)
                nc.vector.reciprocal(out=rZ_sb[:, jt:jt + 1],
                                     in_=Z_sb[:, jt:jt + 1])
                a = ph1u_p.tile([P, XL], bf16, tag="a", name=f"a{jt}")
                nc.vector.tensor_scalar_mul(a, u, rZ_sb[:, jt:jt + 1])
                nc.sync.dma_start(out=uTall[:, :, jt, :], in_=a,
                                  transpose=True)
            ph1u.__exit__(None, None, None)
            pse_cm.__exit__(None, None, None)
            ph1_cm.__exit__(None, None, None)

            # ---- b_att / x2y / w14 chain (overlaps y2x phase) ----
            ph2_cm = tc.tile_pool(name="ph2", bufs=1)
            ph2 = ph2_cm.__enter__()
            psy_cm = tc.tile_pool(name="psy", bufs=2, space="PSUM")
            psy = psy_cm.__enter__()
            psp_cm = tc.tile_pool(name="psp", bufs=2, space="PSUM")
            psp = psp_cm.__enter__()

            bexp = small.tile([P, NJT], f32, tag="bexp")
            nc.vector.tensor_mul(bexp, m_sb, ec_sb)
            # 1/sum_j bexp  (folded into w14 later)
            s1 = small.tile([P, 1], f32, tag="s1")
            nc.vector.tensor_reduce(out=s1, in_=bexp, axis=AXX, op=ADD)
            tps = pst.tile([1, P], f32, tag="pst", name="tps")
            nc.tensor.transpose(tps, s1, ident32)
            s1r = small.tile([1, P], f32, tag="s1r")
            nc.vector.tensor_copy(out=s1r, in_=tps)
            s0 = small.tile([1, 1], f32, tag="s0")
            nc.vector.tensor_reduce(out=s0, in_=s1r, axis=AXX, op=ADD)
            rb0 = small.tile([1, 1], f32, tag="rb0")
            nc.vector.reciprocal(rb0, s0)
            rbp = pst.tile([P, 1], f32, tag="pst", name="rbp")
            nc.tensor.matmul(rbp, ones32, rb0, start=True, stop=True,
                             skip_group_check=True)
            rbz = small.tile([P, 1], f32, tag="rbz")
            nc.vector.tensor_copy(out=rbz, in_=rbp)

            # bexp as a broadcast row: transpose + DRAM bounce + stride-0 read
            btp = pst.tile([NJT, P], f32, tag="pst", name="btp")
            nc.tensor.transpose(btp, bexp, ident32)
            btmp = small.tile([NJT, P], bf16, tag="btmp")
            nc.vector.tensor_copy(out=btmp, in_=btp)
            scrd = dpool.tile([1, YL], bf16, tag="scrd")
            nc.sync.dma_start(out=scrd, in_=btmp)
            bexpb = ph2.tile([P, YL], bf16, tag="bexpb")
            nc.sync.dma_start(out=bexpb, in_=bass.AP(
                tensor=scrd.tensor, offset=scrd.offset, ap=[[0, P], [1, YL]]))

            # x2y_unnorm^T[h] = sum_j e2^T[h,j] * bexp[j]  (vector mul+reduce)
            x2u = small.tile([P, NHC], f32, tag="x2u")
            ph2t = tc.tile_pool(name="ph2t", bufs=2)
            ph2t_p = ph2t.__enter__()
            for hc in range(NHC):
                t6 = ph2t_p.tile([P, YL], bf16, tag="t6", name=f"t6_{hc}")
                nc.vector.tensor_mul(t6, e2tt[:, hc, :], bexpb)
                nc.vector.tensor_reduce(out=x2u[:, hc:hc + 1], in_=t6,
                                        axis=AXX, op=ADD)
            ph2t.__exit__(None, None, None)
            x2yn = small.tile([P, NHC], f32, tag="x2yn")
            nc.vector.tensor_scalar_mul(x2yn, x2u, rbz)
            # w14 = w1 + x2y*w4
            w14 = res.tile([P, NHC, OUT], bf16, tag="w14")
            for hc in range(NHC):
                nc.vector.scalar_tensor_tensor(
                    out=w14[:, hc, :], in0=w4b[:, hc, :],
                    scalar=x2yn[:, hc:hc + 1], in1=w1b[:, hc, :],
                    op0=MUL, op1=ADD)

            # ---- phase 2: y2x (all j, N=512) + fused reduction pass ----
            y2xT = ph2.tile([P, NHC, YL], bf16, tag="y2xT")
            bl3 = ph2.tile([P, NHC, YL], bf16, tag="bl3")
            for jw in range(NJW):
                wsl = slice(jw * 512, (jw + 1) * 512)
                for hc in range(NHC):
                    yp = psy.tile([P, 512], f32, tag="psy",
                                  name=f"yp{jw}_{hc}")
                    for d in range(NIC):
                        nc.tensor.matmul(
                            yp, e1nb[:, d, hc * P:(hc + 1) * P],
                            uTall[:, d, jw * 4:(jw + 1) * 4, :],
                            start=(d == 0), stop=(d == NIC - 1))
                    nc.vector.tensor_copy(out=y2xT[:, hc, wsl], in_=yp)
                    nc.vector.tensor_mul(bl3[:, hc, wsl], e2tt[:, hc, wsl],
                                         y2xT[:, hc, wsl])
                # fused pass for this window's 4 j-tiles
                for jt in range(jw * 4, (jw + 1) * 4):
                    jsl = slice(jt * P, (jt + 1) * P)
                    op = psp.tile([P, OUT], f32, tag="psp", name=f"op{jt}")
                    for hc in range(NHC):
                        nc.tensor.matmul(op, y2xT[:, hc, jsl], w2b[:, hc, :],
                                         start=(hc == 0), stop=False,
                                         skip_group_check=True)
                    for hc in range(NHC):
                        nc.tensor.matmul(op, bl3[:, hc, jsl], w3b[:, hc, :],
                                         start=False, stop=False,
                                         skip_group_check=True)
                    for hc in range(NHC):
                        nc.tensor.matmul(op, e2tt[:, hc, jsl], w14[:, hc, :],
                                         start=False, stop=(hc == NHC - 1),
                                         skip_group_check=True)
                    fin = stage.tile([P, OUT], f32, tag="fin", name=f"fin{jt}")
                    nc.vector.tensor_add(fin, op, bred_bc)
                    nc.sync.dma_start(out=out_d[jsl, :], in_=fin)
            psp_cm.__exit__(None, None, None)
            psy_cm.__exit__(None, None, None)
            ph2_cm.__exit__(None, None, None)

    return nc


def _get_nc(drain_fix=True):
    if "nc" not in _CACHE:
        _CACHE["nc"] = _build()
    if drain_fix and not _CACHE.get("drain_fixed"):
        import concourse.mybir as mybir
        _fix_waits(_CACHE["nc"], mybir, max_waits=1)
        _CACHE["drain_fixed"] = True
    return _CACHE["nc"]


def _prep(emb1, emb2, w_c, b_c, w_q, b_q, w_cq, b_cq, w_red, b_red):
    """Host-side prep: returns per-core input maps."""
    bf = ml_dtypes.bfloat16
    emb1 = np.asarray(emb1, dtype=np.float32)
    emb2 = np.asarray(emb2, dtype=np.float32)
    w_cq = np.asarray(w_cq, dtype=np.float32)
    w_q = np.asarray(w_q, dtype=np.float32)
    w_c = np.asarray(w_c, dtype=np.float32)
    w_red = np.asarray(w_red, dtype=np.float32)

    e1t = np.ascontiguousarray((emb1 * w_cq).transpose(0, 2, 1).astype(bf))
    e2t = np.ascontiguousarray(emb2.transpose(0, 2, 1).astype(bf))
    e1n = np.ascontiguousarray(emb1.astype(bf))
    # biases: b_c cancels in both softmaxes; b_q + b_cq shift every s entry
    # equally -> cancel too, but fold into q for generality (free).
    q = (emb1 @ w_q + float(b_q) + float(b_cq)).astype(np.float32)  # (B, XL)
    c = (emb2 @ w_c).astype(np.float32)                             # (B, YL)
    ec = np.exp(c - c.max(axis=1, keepdims=True))
    ec_pt = np.ascontiguousarray(
        ec.reshape(-1, NJT, P).transpose(0, 2, 1))                  # (B, P, NJT)

    w1 = np.ascontiguousarray(w_red[0:H].astype(bf))
    w2 = np.ascontiguousarray(w_red[H:2 * H].astype(bf))
    w3 = np.ascontiguousarray(w_red[2 * H:3 * H].astype(bf))
    w4 = np.ascontiguousarray(w_red[3 * H:4 * H].astype(bf))
    bred = np.ascontiguousarray(np.asarray(b_red, np.float32).reshape(1, OUT))

    in_maps = []
    for b in range(NCORES):
        in_maps.append({
            "e1t": e1t[b], "e1n": e1n[b], "e2t": e2t[b],
            "q": np.ascontiguousarray(q[b].reshape(1, XL)),
            "ec": np.ascontiguousarray(ec_pt[b]),
            "w1": w1, "w2": w2, "w3": w3, "w4": w4, "bred": bred,
        })
    return in_maps


def kernel(emb1, emb2, w_c, b_c, w_q, b_q, w_cq, b_cq, w_red, b_red):
    from concourse.bass_utils import run_bass_kernel_spmd

    nc = _get_nc()
    in_maps = _prep(emb1, emb2, w_c, b_c, w_q, b_q, w_cq, b_cq, w_red, b_red)
    res = run_bass_kernel_spmd(nc, in_maps, core_ids=list(range(NCORES)))
    return np.stack([res.results[i]["out"] for i in range(NCORES)], axis=0)
